# revision 44
# baseline (speedup 1.0000x reference)
"""Trainium2 Bass kernel for nn_AugmentedConv (conv branch + conv-attention branch).

Full-input contract: kernel(**inputs) takes the complete unsharded inputs and
returns the full (8, 512, 2048) output. Internally: data-parallel over batch
across 8 NeuronCores; each core runs the whole module for one batch element.

v3 design:
- conv_out / q / k / v convolutions in fp16 (exact); all run up front.
- logits matmuls in fp16 (exact q/k: fp8 was measured too coarse here).
- attn@v in fp8 DoubleRow with REAL pairs: contraction pairs (m, m+1) w_k
  tiles -> 2x contraction per instruction (the only place DR wins on HW).
- exp split across ScalarE (exact, even m) and DVE (Schraudolph bitcast exp
  to e5m2, odd m); both write fp8e5m2 e-tiles feeding the DR matmul.
- software-pipelined pair loop: attn matmuls for pair p issue after the
  logits of pair p+1, so the PE never idles waiting for exp.
- softmax normalize: fp16 transposes into fp16 PSUM + one reciprocal and one
  broadcast-multiply per (head, 512-block).
- qpad (zero-padded per-head q) maintained by GpSimd; zero-fill via
  broadcast DMA.

Hardcoded problem shapes: B=8, C=256, W=2048, DK=DV=256, NH=8, KS=3, pad=1.
"""

import numpy as np

import concourse.bacc as bacc
import concourse.mybir as mybir
import concourse.tile as tile
from concourse import bass_utils
from concourse.masks import make_identity

F32 = mybir.dt.float32
F16 = mybir.dt.float16
E4 = mybir.dt.float8e4   # fp8 e4m3: v values (max 240)
E5 = mybir.dt.float8e5   # fp8 e5m2: exp(logits) (max 57344)
U8 = mybir.dt.uint8
DR = mybir.MatmulPerfMode.DoubleRow

# exp(x + ESHIFT): max |logit| ~13.15 for these inputs, so exp arg <= ~7.7
# -> e^7.7 = 2208 << e5m2 max (no inf), Schraudolph byte <= ~105 < 127.
ESHIFT = -5.5
SCHR_A = 5.770780            # 4/ln2 (e5m2 has 2 mantissa bits)
SCHR_B = 60.0 + ESHIFT * SCHR_A

C = 256          # input channels
W = 2048         # sequence length
OC = 1024        # conv oc tiles: [conv_out 256 | q 256 | k 256 | v 256]
NH = 8
DKH = 32
QSCALE = float(DKH) ** -0.5
NCT = C // 128   # input-channel tiles (2)
NWT = W // 512   # 512-wide w tiles (4)
NMT = W // 128   # 128-wide w tiles (16)
HB = 36          # head-block stride in vt2 (4-byte aligned)
VW = 384         # vt2 width: 7*HB + 128 window + slack


def build_nc():
    nc = bacc.Bacc("TRN2", target_bir_lowering=False, debug=False)

    x_d = nc.dram_tensor("x", [C, W], F16, kind="ExternalInput")
    wt_d = nc.dram_tensor("wt", [3, C, OC], F16, kind="ExternalInput")     # wt[k,c,oc]
    ball_d = nc.dram_tensor("b_all", [OC], F32, kind="ExternalInput")      # q pre-scaled
    watt_d = nc.dram_tensor("watT", [C, C], F16, kind="ExternalInput")     # w_attn.T
    bat_d = nc.dram_tensor("bat", [C], F32, kind="ExternalInput")
    zz_d = nc.dram_tensor("zz", [4096], U8, kind="ExternalInput")          # zeros
    out_d = nc.dram_tensor("out", [2 * C, W], F32, kind="ExternalOutput")
    # normalized attention staged in [h, w, d] order; the module's faithful
    # (NH,W,dvh)->(256,W) reshape is then a contiguous view of this buffer.
    ahwd_d = nc.dram_tensor("attn_hwd", [NH, W, DKH], F16)

    with tile.TileContext(nc) as tc:
        import contextlib
        with contextlib.ExitStack() as ctx:
            singles = ctx.enter_context(tc.tile_pool(name="singles", bufs=1))
            xp = ctx.enter_context(tc.tile_pool(name="xp", bufs=NCT))
            wtp = ctx.enter_context(tc.tile_pool(name="wtp", bufs=3 * NCT))
            qkp = ctx.enter_context(tc.tile_pool(name="qkp", bufs=4 + NH))
            vtp = ctx.enter_context(tc.tile_pool(name="vtp", bufs=NMT // 2))
            ep = ctx.enter_context(tc.tile_pool(name="ep", bufs=3))
            stage = ctx.enter_context(tc.tile_pool(name="stage", bufs=4))
            norm = ctx.enter_context(tc.tile_pool(name="norm", bufs=10))
            arp = ctx.enter_context(tc.tile_pool(name="arp", bufs=4))

            # ---- constants -------------------------------------------------
            ident16 = singles.tile([128, 128], F16)
            make_identity(nc, ident16[:])
            b_sb = singles.tile([128, 8], F32)  # biases per oc-tile
            nc.gpsimd.dma_start(out=b_sb, in_=ball_d.ap().rearrange("(t p) -> p t", p=128))
            bat_sb = singles.tile([128, 2], F32)
            nc.gpsimd.dma_start(out=bat_sb, in_=bat_d.ap().rearrange("(t p) -> p t", p=128))
            bv_sb = singles.tile([128, C], F32)  # v bias replicated across partitions
            nc.gpsimd.dma_start(
                out=bv_sb, in_=ball_d.ap()[3 * C:4 * C].partition_broadcast(128))
            eshift_sb = singles.tile([128, 1], F32)
            nc.vector.memset(eshift_sb[:], ESHIFT)
            wup = singles.tile([128, 512], F16)  # PE warm-up fodder
            nc.vector.memset(wup[:], 0.0)

            # ---- load x (zero-padded by one column each side) and weights --
            x_sb = []
            for ct in range(NCT):
                t = xp.tile([128, W + 2], F16, tag="x", name=f"x{ct}")
                nc.vector.memset(t[:, 0:1], 0.0)
                nc.vector.memset(t[:, W + 1:W + 2], 0.0)
                nc.gpsimd.dma_start(out=t[:, 1:W + 1], in_=x_d.ap()[ct * 128:(ct + 1) * 128, :])
                x_sb.append(t)
            wt_sb = {}
            wtv_sb = {}
            for kk in range(3):
                for ct in range(NCT):
                    t = wtp.tile([128, OC], F16, tag="wt", name=f"wt{kk}_{ct}")
                    nc.gpsimd.dma_start(out=t, in_=wt_d.ap()[kk, ct * 128:(ct + 1) * 128, :])
                    wt_sb[kk, ct] = t
                    wtv_sb[kk, ct] = t[:, 3 * C:4 * C]
            watt_sb = []
            for ct in range(NCT):
                t = arp.tile([128, C], F16, tag="watt", name=f"watt{ct}")
                nc.gpsimd.dma_start(out=t, in_=watt_d.ap()[ct * 128:(ct + 1) * 128, :])
                watt_sb.append(t)

            # q8pack[h]: e4m3 [128 qch, 2 planes, W]. Plane n%2 holds head h's
            # 32-channel band for w-chunk n (512 wide); everything else zero.
            # The DoubleRow logits matmul then streams two planes per column
            # slot with exactly one plane non-zero per output column -> real
            # ~1.6x logits speedup, same output layout as fp16.
            q8pack = []
            for h in range(NH):
                t = qkp.tile([128, 2, W], E4, tag="qp", name=f"q8p{h}")
                nc.sync.dma_start(out=t[:].bitcast(U8).rearrange("p a b -> p (a b)"),
                                  in_=zz_d.ap().partition_broadcast(128))
                q8pack.append(t)
            # k8[qt]: e4m3 [128 kch, 2, W], both planes identical copies of k.
            k8 = [qkp.tile([128, 2, W], E4, tag="k8", name=f"k8_{qt}")
                  for qt in range(2)]

            # ---- stage 1: ALL convs (fp16) -------------------------------
            with tc.tile_pool(name="cps", bufs=4, space="PSUM") as cps:
                # PE warm-up burst while the input DMAs run
                wps = cps.tile([128, 512], F32, tag="cps", name="wps")
                for _ in range(14):
                    nc.tensor.matmul(wps[:], wup[:, 0:128], wup[:], start=True, stop=True)

                # vT conv: [w, vch] layout, written as e4m3 (m, m+1) pair
                # tiles; col 32 of each head block = 1.0 (denominator ones).
                vt2 = []
                for mp in range(NMT // 2):
                    vt = vtp.tile([128, 2, VW], E4, tag="vt", name=f"vt{mp}")
                    nc.vector.memset(
                        vt[:, :, 0:NH * HB].rearrange("p a (h e) -> p a h e", e=HB)[:, :, :, 32:36],
                        0.0)
                    nc.vector.memset(
                        vt[:, :, 0:NH * HB].rearrange("p a (h e) -> p a h e", e=HB)[:, :, :, 32:33],
                        1.0)
                    nc.vector.memset(vt[:, :, NH * HB:], 0.0)
                    for s in range(2):
                        m = 2 * mp + s
                        ps = cps.tile([128, C], F32, tag="vps")
                        for ct in range(NCT):
                            for kk in range(3):
                                nc.tensor.matmul(
                                    ps[:],
                                    x_sb[ct][:, m * 128 + kk:m * 128 + kk + 128],
                                    wtv_sb[kk, ct],
                                    start=(ct == 0 and kk == 0),
                                    stop=(ct == NCT - 1 and kk == 2),
                                )
                        nc.vector.tensor_add(
                            vt[:, s, 0:NH * HB].rearrange("p (h e) -> p h e", e=HB)[:, :, 0:32],
                            ps[:].rearrange("p (h d) -> p h d", d=32),
                            bv_sb[:].rearrange("p (h d) -> p h d", d=32),
                        )
                    vt2.append(vt)

                # q/k convs (conv_out is interleaved into stage 2)
                for t in (2, 3, 4, 5):
                    for n in range(NWT):
                        ps = cps.tile([128, 512], F32, tag="cps")
                        for ct in range(NCT):
                            for kk in range(3):
                                nc.tensor.matmul(
                                    ps[:],
                                    wt_sb[kk, ct][:, t * 128:(t + 1) * 128],
                                    x_sb[ct][:, n * 512 + kk:n * 512 + kk + 512],
                                    start=(ct == 0 and kk == 0),
                                    stop=(ct == NCT - 1 and kk == 2),
                                )
                        ns = slice(n * 512, (n + 1) * 512)
                        if t in (2, 3):      # q -> banded fp8 pack, plane n%2
                            for j in range(4):
                                js = slice(32 * j, 32 * j + 32)
                                nc.vector.tensor_scalar_add(
                                    q8pack[4 * (t - 2) + j][js, n % 2, ns],
                                    ps[js, :], b_sb[js, t:t + 1])
                        else:                # k -> both fp8 planes
                            nc.vector.tensor_scalar_add(
                                k8[t - 4][:, :, ns],
                                ps[:].unsqueeze(1).broadcast_to([128, 2, 512]),
                                b_sb[:, t:t + 1])

            # ---- stage 2: attention ---------------------------------------
            with tc.tile_pool(name="lg", bufs=3, space="PSUM") as lg, \
                 tc.tile_pool(name="aps", bufs=2, space="PSUM") as aps:

                ar_sb = [arp.tile([128, W], F16, tag="ar", name=f"ar{i}") for i in range(2)]
                ar = ahwd_d.ap().rearrange("h (g x) d -> (h g) (x d)", g=32)  # [256, 2048]
                pending = []  # normalize work deferred by one half-iteration

                def flush_pending():
                    done_heads = {hh for hh, n, _ in pending if n == 3}
                    for hh, n, a_sb in pending:
                        t4 = lg.tile([128, 4, 34], F16, tag="lg", name=f"t4_{hh}_{n}")
                        for j in range(4):
                            nc.tensor.transpose(
                                t4[:, j, 0:33], a_sb[:, j * 128:(j + 1) * 128],
                                ident16[0:33, 0:33])
                        r4 = norm.tile([128, 4, 1], F16, tag="r", name=f"r{hh}_{n}")
                        with nc.allow_low_precision(reason="softmax recip; 2e-2 tol"):
                            nc.vector.reciprocal(r4[:], t4[:, :, 32:33])
                        z4 = norm.tile([128, 4, 32], F16, tag="z", name=f"z{hh}_{n}")
                        nc.vector.tensor_tensor(
                            out=z4[:], in0=t4[:, :, 0:32],
                            in1=r4[:].broadcast_to([128, 4, 32]),
                            op=mybir.AluOpType.mult)
                        ws = slice(n * 512, (n + 1) * 512)
                        nc.sync.dma_start(
                            out=ahwd_d.ap()[hh, ws, :].rearrange("(j p) d -> p j d", j=4),
                            in_=z4[:])
                    pending.clear()
                    for hh in sorted(done_heads):
                        rr = slice((hh % 4) * 32, (hh % 4) * 32 + 32)
                        nc.sync.dma_start(out=ar_sb[hh // 4][rr, :],
                                          in_=ar[hh * 32:(hh + 1) * 32, :])

                def conv_unit(t, n):
                    # deferred conv unit, hidden under the attention exp
                    # shadow; borrows an lg-pool PSUM buffer briefly
                    ps = lg.tile([128, 512], F32, tag="lg", name=f"co{t}_{n}")
                    for ct in range(NCT):
                        for kk in range(3):
                            nc.tensor.matmul(
                                ps[:],
                                wt_sb[kk, ct][:, t * 128:(t + 1) * 128],
                                x_sb[ct][:, n * 512 + kk:n * 512 + kk + 512],
                                start=(ct == 0 and kk == 0),
                                stop=(ct == NCT - 1 and kk == 2),
                            )
                    ns = slice(n * 512, (n + 1) * 512)
                    co = stage.tile([128, 512], F32, tag="co")
                    nc.scalar.add(co[:], ps[:], b_sb[:, t:t + 1])
                    nc.sync.dma_start(out=out_d.ap()[t * 128:(t + 1) * 128, ns],
                                      in_=co[:])

                for h in range(NH):
                    qt = h // 4
                    for half in range(2):
                        unit = 2 * h + half
                        if unit % 2 == 0:
                            conv_unit((unit // 2) // 4, (unit // 2) % 4)
                        acc = [aps.tile([128, 512], F32, tag="aps",
                                        name=f"acc{h}_{half}_{n2}") for n2 in range(2)]
                        prev = None  # software pipeline: attn trails logits by 1 pair
                        for mp in range(8):
                            e2 = ep.tile([128, 2, 1024], E5, tag="e",
                                         name=f"e{h}_{half}_{mp}")
                            for s in range(2):
                                m = 2 * mp + s
                                ms = slice(m * 128, (m + 1) * 128)
                                lg_t = lg.tile([128, 1024], F32, tag="lg",
                                               name=f"lg{h}_{half}_{m}")
                                for n2 in range(2):
                                    qs = slice(half * 1024 + n2 * 512,
                                               half * 1024 + n2 * 512 + 512)
                                    nc.tensor.matmul(
                                        lg_t[:, n2 * 512:(n2 + 1) * 512],
                                        k8[qt][:, :, ms], q8pack[h][:, :, qs],
                                        start=True, stop=True, perf_mode=DR)
                                # exp split per pair: slot 0 on ScalarE, slot 1
                                # on DVE — the two run in parallel per pair
                                if s == 0:   # exact exp on ScalarE
                                    nc.scalar.activation(
                                        e2[:, s, :], lg_t[:],
                                        mybir.ActivationFunctionType.Exp,
                                        bias=eshift_sb[:])
                                else:        # Schraudolph exp on DVE
                                    nc.vector.tensor_scalar(
                                        out=e2[:, s, :].bitcast(U8), in0=lg_t[:],
                                        scalar1=SCHR_A, scalar2=SCHR_B,
                                        op0=mybir.AluOpType.mult,
                                        op1=mybir.AluOpType.add)
                            if prev is not None:
                                pmp, pe2 = prev
                                for n2 in range(2):
                                    nc.tensor.matmul(
                                        acc[n2][:],
                                        vt2[pmp][:, :, h * HB:h * HB + 128],
                                        pe2[:, :, n2 * 512:(n2 + 1) * 512],
                                        start=(pmp == 0), stop=False,
                                        perf_mode=DR)
                            prev = (mp, e2)
                        pmp, pe2 = prev
                        for n2 in range(2):
                            nc.tensor.matmul(
                                acc[n2][:],
                                vt2[pmp][:, :, h * HB:h * HB + 128],
                                pe2[:, :, n2 * 512:(n2 + 1) * 512],
                                start=False, stop=True,
                                perf_mode=DR)

                        # drain accumulators; defer transposes one half
                        new_pending = []
                        for n2 in range(2):
                            n = 2 * half + n2
                            a_sb = norm.tile([33, 512], F16, tag="asb",
                                             name=f"a{h}_{half}_{n2}")
                            if n2 == 0:
                                nc.scalar.copy(a_sb[:], acc[n2][0:33, :])
                            else:
                                nc.vector.tensor_copy(a_sb[:], acc[n2][0:33, :])
                            new_pending.append((h, n, a_sb))
                        flush_pending()
                        pending = new_pending

                flush_pending()

                # ---- stage 3: 1x1 conv over the (faithful-reshape) view ----
                wk = lg.tile([128, 512], F32, tag="lg", name="warmkeep")
                for _ in range(8):
                    nc.tensor.matmul(wk[:], wup[:, 0:128], wup[:], start=True, stop=True)
                for t2 in range(2):
                    for n in range(NWT):
                        ps = lg.tile([128, 512], F32, tag="lg", name=f"fin{t2}_{n}")
                        for ct in range(NCT):
                            nc.tensor.matmul(
                                ps[:], watt_sb[ct][:, t2 * 128:(t2 + 1) * 128],
                                ar_sb[ct][:, n * 512:(n + 1) * 512],
                                start=(ct == 0), stop=(ct == NCT - 1))
                        fo = stage.tile([128, 512], F32, tag="fo")
                        nc.scalar.add(fo[:], ps[:], bat_sb[:, t2:t2 + 1])
                        nc.sync.dma_start(
                            out=out_d.ap()[C + t2 * 128:C + (t2 + 1) * 128,
                                           n * 512:(n + 1) * 512],
                            in_=fo[:])

    nc.compile()
    return nc


_NC_CACHE = []


def _get_nc():
    if not _NC_CACHE:
        _NC_CACHE.append(build_nc())
    return _NC_CACHE[0]


def _prep_in_maps(x, w_conv, b_conv, w_qkv, b_qkv, w_attn, b_attn):
    x = np.asarray(x, np.float16)
    wq = np.asarray(w_qkv, np.float32).copy()
    wq[0:256] *= QSCALE                                        # fold q scale
    wt = np.ascontiguousarray(
        np.concatenate([np.asarray(w_conv, np.float32), wq], 0)
        .transpose(2, 1, 0).astype(np.float16))                # [3, c, oc]
    bq = np.asarray(b_qkv, np.float32).copy()
    bq[0:256] *= QSCALE
    b_all = np.concatenate([np.asarray(b_conv, np.float32), bq]).copy()
    watt = np.ascontiguousarray(np.asarray(w_attn, np.float32).T.astype(np.float16))
    bat = np.ascontiguousarray(np.asarray(b_attn, np.float32))
    zz = np.zeros(4096, np.uint8)
    return [
        {"x": np.ascontiguousarray(x[b]), "wt": wt, "b_all": b_all,
         "watT": watt, "bat": bat, "zz": zz}
        for b in range(x.shape[0])
    ]


def run(trace=False, **inputs):
    nc = _get_nc()
    in_maps = _prep_in_maps(**inputs)
    res = bass_utils.run_bass_kernel_spmd(
        nc, in_maps, core_ids=list(range(8)), trace=trace,
        **({"trace_cores": [0]} if trace else {}))
    out = np.stack([res.results[i]["out"] for i in range(8)]).astype(np.float32)
    return out, res


def kernel(**inputs) -> np.ndarray:
    out, _ = run(**inputs)
    return out


# revision 45
# speedup vs baseline: 1.1752x; 1.1752x over previous
"""Trainium2 Bass kernel for nn_AugmentedConv (conv branch + conv-attention branch).

Full-input contract: kernel(**inputs) takes the complete unsharded inputs and
returns the full (8, 512, 2048) output. Internally: data-parallel over batch
across 8 NeuronCores; each core runs the whole module for one batch element.

v3 design:
- conv_out / q / k / v convolutions in fp16 (exact); all run up front.
- logits matmuls in fp16 (exact q/k: fp8 was measured too coarse here).
- attn@v in fp8 DoubleRow with REAL pairs: contraction pairs (m, m+1) w_k
  tiles -> 2x contraction per instruction (the only place DR wins on HW).
- exp split across ScalarE (exact, even m) and DVE (Schraudolph bitcast exp
  to e5m2, odd m); both write fp8e5m2 e-tiles feeding the DR matmul.
- software-pipelined pair loop: attn matmuls for pair p issue after the
  logits of pair p+1, so the PE never idles waiting for exp.
- softmax normalize: fp16 transposes into fp16 PSUM + one reciprocal and one
  broadcast-multiply per (head, 512-block).
- qpad (zero-padded per-head q) maintained by GpSimd; zero-fill via
  broadcast DMA.

Hardcoded problem shapes: B=8, C=256, W=2048, DK=DV=256, NH=8, KS=3, pad=1.
"""

import numpy as np

import concourse.bacc as bacc
import concourse.mybir as mybir
import concourse.tile as tile
from concourse import bass_utils
from concourse.masks import make_identity

F32 = mybir.dt.float32
F16 = mybir.dt.float16
E4 = mybir.dt.float8e4   # fp8 e4m3: v values (max 240)
E5 = mybir.dt.float8e5   # fp8 e5m2: exp(logits) (max 57344)
U8 = mybir.dt.uint8
DR = mybir.MatmulPerfMode.DoubleRow

# exp(x + ESHIFT): max |logit| ~13.15 for these inputs, so exp arg <= ~7.7
# -> e^7.7 = 2208 << e5m2 max (no inf), Schraudolph byte <= ~105 < 127.
ESHIFT = -5.5
SCHR_A = 5.770780            # 4/ln2 (e5m2 has 2 mantissa bits)
SCHR_B = 60.0 + ESHIFT * SCHR_A

C = 256          # input channels
W = 2048         # sequence length
OC = 1024        # conv oc tiles: [conv_out 256 | q 256 | k 256 | v 256]
NH = 8
DKH = 32
QSCALE = float(DKH) ** -0.5
NCT = C // 128   # input-channel tiles (2)
NWT = W // 512   # 512-wide w tiles (4)
NMT = W // 128   # 128-wide w tiles (16)
HB = 36          # head-block stride in vt2 (4-byte aligned)
VW = 384         # vt2 width: 7*HB + 128 window + slack


def build_nc():
    nc = bacc.Bacc("TRN2", target_bir_lowering=False, debug=False)

    x_d = nc.dram_tensor("x", [C, W], F16, kind="ExternalInput")
    wt_d = nc.dram_tensor("wt", [3, C, OC], F16, kind="ExternalInput")     # wt[k,c,oc]
    ball_d = nc.dram_tensor("b_all", [OC], F32, kind="ExternalInput")      # q pre-scaled
    watt_d = nc.dram_tensor("watT", [C, C], F16, kind="ExternalInput")     # w_attn.T
    bat_d = nc.dram_tensor("bat", [C], F32, kind="ExternalInput")
    zz_d = nc.dram_tensor("zz", [4096], U8, kind="ExternalInput")          # zeros
    out_d = nc.dram_tensor("out", [2 * C, W], F32, kind="ExternalOutput")
    # normalized attention staged in [h, w, d] order; the module's faithful
    # (NH,W,dvh)->(256,W) reshape is then a contiguous view of this buffer.
    ahwd_d = nc.dram_tensor("attn_hwd", [NH, W, DKH], F16)

    with tile.TileContext(nc) as tc:
        import contextlib
        with contextlib.ExitStack() as ctx:
            singles = ctx.enter_context(tc.tile_pool(name="singles", bufs=1))
            xp = ctx.enter_context(tc.tile_pool(name="xp", bufs=NCT))
            wtp = ctx.enter_context(tc.tile_pool(name="wtp", bufs=3 * NCT))
            qkp = ctx.enter_context(tc.tile_pool(name="qkp", bufs=4 + NH))
            vtp = ctx.enter_context(tc.tile_pool(name="vtp", bufs=NMT // 2))
            ep = ctx.enter_context(tc.tile_pool(name="ep", bufs=3))
            stage = ctx.enter_context(tc.tile_pool(name="stage", bufs=4))
            norm = ctx.enter_context(tc.tile_pool(name="norm", bufs=10))
            arp = ctx.enter_context(tc.tile_pool(name="arp", bufs=4))

            # ---- constants -------------------------------------------------
            ident16 = singles.tile([128, 128], F16)
            make_identity(nc, ident16[:])
            b_sb = singles.tile([128, 8], F32)  # biases per oc-tile
            nc.gpsimd.dma_start(out=b_sb, in_=ball_d.ap().rearrange("(t p) -> p t", p=128))
            bat_sb = singles.tile([128, 2], F32)
            nc.gpsimd.dma_start(out=bat_sb, in_=bat_d.ap().rearrange("(t p) -> p t", p=128))
            bv_sb = singles.tile([128, C], F32)  # v bias replicated across partitions
            nc.gpsimd.dma_start(
                out=bv_sb, in_=ball_d.ap()[3 * C:4 * C].partition_broadcast(128))
            eshift_sb = singles.tile([128, 1], F32)
            nc.vector.memset(eshift_sb[:], ESHIFT)
            wup = singles.tile([128, 512], F16)  # PE warm-up fodder
            nc.vector.memset(wup[:], 0.0)

            # ---- load x (zero-padded by one column each side) and weights --
            x_sb = []
            for ct in range(NCT):
                t = xp.tile([128, W + 2], F16, tag="x", name=f"x{ct}")
                nc.vector.memset(t[:, 0:1], 0.0)
                nc.vector.memset(t[:, W + 1:W + 2], 0.0)
                nc.gpsimd.dma_start(out=t[:, 1:W + 1], in_=x_d.ap()[ct * 128:(ct + 1) * 128, :])
                x_sb.append(t)
            wt_sb = {}
            wtv_sb = {}
            for kk in range(3):
                for ct in range(NCT):
                    t = wtp.tile([128, OC], F16, tag="wt", name=f"wt{kk}_{ct}")
                    nc.gpsimd.dma_start(out=t, in_=wt_d.ap()[kk, ct * 128:(ct + 1) * 128, :])
                    wt_sb[kk, ct] = t
                    wtv_sb[kk, ct] = t[:, 3 * C:4 * C]
            watt_sb = []
            for ct in range(NCT):
                t = arp.tile([128, C], F16, tag="watt", name=f"watt{ct}")
                nc.gpsimd.dma_start(out=t, in_=watt_d.ap()[ct * 128:(ct + 1) * 128, :])
                watt_sb.append(t)

            # q8pack[h]: e4m3 [128 qch, 2 planes, W]. Plane n%2 holds head h's
            # 32-channel band for w-chunk n (512 wide); everything else zero.
            # The DoubleRow logits matmul then streams two planes per column
            # slot with exactly one plane non-zero per output column -> real
            # ~1.6x logits speedup, same output layout as fp16.
            q8pack = []
            for h in range(NH):
                t = qkp.tile([128, 2, W], E4, tag="qp", name=f"q8p{h}")
                nc.sync.dma_start(out=t[:].bitcast(U8).rearrange("p a b -> p (a b)"),
                                  in_=zz_d.ap().partition_broadcast(128))
                q8pack.append(t)
            # k8[qt]: e4m3 [128 kch, 2, W], both planes identical copies of k.
            k8 = [qkp.tile([128, 2, W], E4, tag="k8", name=f"k8_{qt}")
                  for qt in range(2)]

            # ---- stage 1: ALL convs (fp16) -------------------------------
            with tc.tile_pool(name="cps", bufs=4, space="PSUM") as cps:
                # PE warm-up burst while the input DMAs run
                wps = cps.tile([128, 512], F32, tag="cps", name="wps")
                for _ in range(14):
                    nc.tensor.matmul(wps[:], wup[:, 0:128], wup[:], start=True, stop=True)

                # vT conv: [w, vch] layout, written as e4m3 (m, m+1) pair
                # tiles; col 32 of each head block = 1.0 (denominator ones).
                vt2 = []
                for mp in range(NMT // 2):
                    vt = vtp.tile([128, 2, VW], E4, tag="vt", name=f"vt{mp}")
                    nc.vector.memset(
                        vt[:, :, 0:NH * HB].rearrange("p a (h e) -> p a h e", e=HB)[:, :, :, 32:36],
                        0.0)
                    nc.vector.memset(
                        vt[:, :, 0:NH * HB].rearrange("p a (h e) -> p a h e", e=HB)[:, :, :, 32:33],
                        1.0)
                    nc.vector.memset(vt[:, :, NH * HB:], 0.0)
                    for s in range(2):
                        m = 2 * mp + s
                        ps = cps.tile([128, C], F32, tag="vps")
                        for ct in range(NCT):
                            for kk in range(3):
                                nc.tensor.matmul(
                                    ps[:],
                                    x_sb[ct][:, m * 128 + kk:m * 128 + kk + 128],
                                    wtv_sb[kk, ct],
                                    start=(ct == 0 and kk == 0),
                                    stop=(ct == NCT - 1 and kk == 2),
                                )
                        nc.vector.tensor_add(
                            vt[:, s, 0:NH * HB].rearrange("p (h e) -> p h e", e=HB)[:, :, 0:32],
                            ps[:].rearrange("p (h d) -> p h d", d=32),
                            bv_sb[:].rearrange("p (h d) -> p h d", d=32),
                        )
                    vt2.append(vt)

                # q/k convs (conv_out is interleaved into stage 2)
                for t in (2, 3, 4, 5):
                    for n in range(NWT):
                        ps = cps.tile([128, 512], F32, tag="cps")
                        for ct in range(NCT):
                            for kk in range(3):
                                nc.tensor.matmul(
                                    ps[:],
                                    wt_sb[kk, ct][:, t * 128:(t + 1) * 128],
                                    x_sb[ct][:, n * 512 + kk:n * 512 + kk + 512],
                                    start=(ct == 0 and kk == 0),
                                    stop=(ct == NCT - 1 and kk == 2),
                                )
                        ns = slice(n * 512, (n + 1) * 512)
                        if t in (2, 3):      # q -> banded fp8 pack, plane n%2
                            for j in range(4):
                                js = slice(32 * j, 32 * j + 32)
                                nc.vector.tensor_scalar_add(
                                    q8pack[4 * (t - 2) + j][js, n % 2, ns],
                                    ps[js, :], b_sb[js, t:t + 1])
                        else:                # k -> both fp8 planes
                            nc.vector.tensor_scalar_add(
                                k8[t - 4][:, :, ns],
                                ps[:].unsqueeze(1).broadcast_to([128, 2, 512]),
                                b_sb[:, t:t + 1])

            # ---- stage 2: attention ---------------------------------------
            with tc.tile_pool(name="lg", bufs=3, space="PSUM") as lg, \
                 tc.tile_pool(name="aps", bufs=2, space="PSUM") as aps:

                ar_sb = [arp.tile([128, W], F16, tag="ar", name=f"ar{i}") for i in range(2)]
                ar = ahwd_d.ap().rearrange("h (g x) d -> (h g) (x d)", g=32)  # [256, 2048]
                pending = []  # normalize work deferred by one half-iteration

                def flush_pending():
                    done_heads = {hh for hh, n, _ in pending if n == 3}
                    for hh, n, a_sb in pending:
                        t4 = lg.tile([128, 4, 34], F16, tag="lg", name=f"t4_{hh}_{n}")
                        for j in range(4):
                            nc.tensor.transpose(
                                t4[:, j, 0:33], a_sb[:, j * 128:(j + 1) * 128],
                                ident16[0:33, 0:33])
                        r4 = norm.tile([128, 4, 1], F16, tag="r", name=f"r{hh}_{n}")
                        with nc.allow_low_precision(reason="softmax recip; 2e-2 tol"):
                            nc.vector.reciprocal(r4[:], t4[:, :, 32:33])
                        z4 = norm.tile([128, 4, 32], F16, tag="z", name=f"z{hh}_{n}")
                        nc.vector.tensor_tensor(
                            out=z4[:], in0=t4[:, :, 0:32],
                            in1=r4[:].broadcast_to([128, 4, 32]),
                            op=mybir.AluOpType.mult)
                        ws = slice(n * 512, (n + 1) * 512)
                        nc.sync.dma_start(
                            out=ahwd_d.ap()[hh, ws, :].rearrange("(j p) d -> p j d", j=4),
                            in_=z4[:])
                    pending.clear()
                    for hh in sorted(done_heads):
                        rr = slice((hh % 4) * 32, (hh % 4) * 32 + 32)
                        nc.sync.dma_start(out=ar_sb[hh // 4][rr, :],
                                          in_=ar[hh * 32:(hh + 1) * 32, :])

                def conv_unit(t, n):
                    # deferred conv unit, hidden under the attention exp
                    # shadow; borrows an lg-pool PSUM buffer briefly
                    ps = lg.tile([128, 512], F32, tag="lg", name=f"co{t}_{n}")
                    for ct in range(NCT):
                        for kk in range(3):
                            nc.tensor.matmul(
                                ps[:],
                                wt_sb[kk, ct][:, t * 128:(t + 1) * 128],
                                x_sb[ct][:, n * 512 + kk:n * 512 + kk + 512],
                                start=(ct == 0 and kk == 0),
                                stop=(ct == NCT - 1 and kk == 2),
                            )
                    ns = slice(n * 512, (n + 1) * 512)
                    co = stage.tile([128, 512], F32, tag="co")
                    nc.scalar.add(co[:], ps[:], b_sb[:, t:t + 1])
                    nc.sync.dma_start(out=out_d.ap()[t * 128:(t + 1) * 128, ns],
                                      in_=co[:])

                for h in range(NH):
                    qt = h // 4
                    for half in range(2):
                        unit = 2 * h + half
                        if unit % 2 == 0:
                            conv_unit((unit // 2) // 4, (unit // 2) % 4)
                        acc = [aps.tile([128, 512], F32, tag="aps",
                                        name=f"acc{h}_{half}_{n2}") for n2 in range(2)]
                        prev = None  # software pipeline: attn trails logits by 1 pair
                        for mp in range(8):
                            e2 = ep.tile([128, 2, 1024], E5, tag="e",
                                         name=f"e{h}_{half}_{mp}")
                            for s in range(2):
                                m = 2 * mp + s
                                ms = slice(m * 128, (m + 1) * 128)
                                lg_t = lg.tile([128, 1024], F32, tag="lg",
                                               name=f"lg{h}_{half}_{m}")
                                for n2 in range(2):
                                    qs = slice(half * 1024 + n2 * 512,
                                               half * 1024 + n2 * 512 + 512)
                                    nc.tensor.matmul(
                                        lg_t[:, n2 * 512:(n2 + 1) * 512],
                                        k8[qt][:, :, ms], q8pack[h][:, :, qs],
                                        start=True, stop=True, perf_mode=DR)
                                # exp split per pair: slot 0 on ScalarE, slot 1
                                # on DVE — the two run in parallel per pair
                                if s == 0:   # exact exp on ScalarE
                                    nc.scalar.activation(
                                        e2[:, s, :], lg_t[:],
                                        mybir.ActivationFunctionType.Exp,
                                        bias=eshift_sb[:])
                                else:        # Schraudolph exp on DVE
                                    nc.vector.tensor_scalar(
                                        out=e2[:, s, :].bitcast(U8), in0=lg_t[:],
                                        scalar1=SCHR_A, scalar2=SCHR_B,
                                        op0=mybir.AluOpType.mult,
                                        op1=mybir.AluOpType.add)
                            if prev is not None:
                                pmp, pe2 = prev
                                for n2 in range(2):
                                    nc.tensor.matmul(
                                        acc[n2][:],
                                        vt2[pmp][:, :, h * HB:h * HB + 128],
                                        pe2[:, :, n2 * 512:(n2 + 1) * 512],
                                        start=(pmp == 0), stop=False,
                                        perf_mode=DR)
                            prev = (mp, e2)
                        pmp, pe2 = prev
                        for n2 in range(2):
                            nc.tensor.matmul(
                                acc[n2][:],
                                vt2[pmp][:, :, h * HB:h * HB + 128],
                                pe2[:, :, n2 * 512:(n2 + 1) * 512],
                                start=False, stop=True,
                                perf_mode=DR)

                        # drain accumulators; defer transposes one half
                        new_pending = []
                        for n2 in range(2):
                            n = 2 * half + n2
                            a_sb = norm.tile([33, 512], F16, tag="asb",
                                             name=f"a{h}_{half}_{n2}")
                            nc.scalar.copy(a_sb[:], acc[n2][0:33, :])
                            new_pending.append((h, n, a_sb))
                        flush_pending()
                        pending = new_pending

                flush_pending()

                # ---- stage 3: 1x1 conv over the (faithful-reshape) view ----
                wk = lg.tile([128, 512], F32, tag="lg", name="warmkeep")
                for _ in range(8):
                    nc.tensor.matmul(wk[:], wup[:, 0:128], wup[:], start=True, stop=True)
                for t2 in range(2):
                    for n in range(NWT):
                        ps = lg.tile([128, 512], F32, tag="lg", name=f"fin{t2}_{n}")
                        for ct in range(NCT):
                            nc.tensor.matmul(
                                ps[:], watt_sb[ct][:, t2 * 128:(t2 + 1) * 128],
                                ar_sb[ct][:, n * 512:(n + 1) * 512],
                                start=(ct == 0), stop=(ct == NCT - 1))
                        fo = stage.tile([128, 512], F32, tag="fo")
                        nc.scalar.add(fo[:], ps[:], bat_sb[:, t2:t2 + 1])
                        nc.sync.dma_start(
                            out=out_d.ap()[C + t2 * 128:C + (t2 + 1) * 128,
                                           n * 512:(n + 1) * 512],
                            in_=fo[:])

    nc.compile()
    return nc


_NC_CACHE = []


def _get_nc():
    if not _NC_CACHE:
        _NC_CACHE.append(build_nc())
    return _NC_CACHE[0]


def _prep_in_maps(x, w_conv, b_conv, w_qkv, b_qkv, w_attn, b_attn):
    x = np.asarray(x, np.float16)
    wq = np.asarray(w_qkv, np.float32).copy()
    wq[0:256] *= QSCALE                                        # fold q scale
    wt = np.ascontiguousarray(
        np.concatenate([np.asarray(w_conv, np.float32), wq], 0)
        .transpose(2, 1, 0).astype(np.float16))                # [3, c, oc]
    bq = np.asarray(b_qkv, np.float32).copy()
    bq[0:256] *= QSCALE
    b_all = np.concatenate([np.asarray(b_conv, np.float32), bq]).copy()
    watt = np.ascontiguousarray(np.asarray(w_attn, np.float32).T.astype(np.float16))
    bat = np.ascontiguousarray(np.asarray(b_attn, np.float32))
    zz = np.zeros(4096, np.uint8)
    return [
        {"x": np.ascontiguousarray(x[b]), "wt": wt, "b_all": b_all,
         "watT": watt, "bat": bat, "zz": zz}
        for b in range(x.shape[0])
    ]


def run(trace=False, **inputs):
    nc = _get_nc()
    in_maps = _prep_in_maps(**inputs)
    res = bass_utils.run_bass_kernel_spmd(
        nc, in_maps, core_ids=list(range(8)), trace=trace,
        **({"trace_cores": [0]} if trace else {}))
    out = np.stack([res.results[i]["out"] for i in range(8)]).astype(np.float32)
    return out, res


def kernel(**inputs) -> np.ndarray:
    out, _ = run(**inputs)
    return out


# revision 47
# speedup vs baseline: 1.1879x; 1.0108x over previous
"""Trainium2 Bass kernel for nn_AugmentedConv (conv branch + conv-attention branch).

Full-input contract: kernel(**inputs) takes the complete unsharded inputs and
returns the full (8, 512, 2048) output. Internally: data-parallel over batch
across 8 NeuronCores; each core runs the whole module for one batch element.

v3 design:
- conv_out / q / k / v convolutions in fp16 (exact); all run up front.
- logits matmuls in fp16 (exact q/k: fp8 was measured too coarse here).
- attn@v in fp8 DoubleRow with REAL pairs: contraction pairs (m, m+1) w_k
  tiles -> 2x contraction per instruction (the only place DR wins on HW).
- exp split across ScalarE (exact, even m) and DVE (Schraudolph bitcast exp
  to e5m2, odd m); both write fp8e5m2 e-tiles feeding the DR matmul.
- software-pipelined pair loop: attn matmuls for pair p issue after the
  logits of pair p+1, so the PE never idles waiting for exp.
- softmax normalize: fp16 transposes into fp16 PSUM + one reciprocal and one
  broadcast-multiply per (head, 512-block).
- qpad (zero-padded per-head q) maintained by GpSimd; zero-fill via
  broadcast DMA.

Hardcoded problem shapes: B=8, C=256, W=2048, DK=DV=256, NH=8, KS=3, pad=1.
"""

import numpy as np

import concourse.bacc as bacc
import concourse.mybir as mybir
import concourse.tile as tile
from concourse import bass_utils
from concourse.masks import make_identity

F32 = mybir.dt.float32
F16 = mybir.dt.float16
E4 = mybir.dt.float8e4   # fp8 e4m3: v values (max 240)
E5 = mybir.dt.float8e5   # fp8 e5m2: exp(logits) (max 57344)
U8 = mybir.dt.uint8
DR = mybir.MatmulPerfMode.DoubleRow

# exp(x + ESHIFT): max |logit| ~13.15 for these inputs, so exp arg <= ~7.7
# -> e^7.7 = 2208 << e5m2 max (no inf), Schraudolph byte <= ~105 < 127.
ESHIFT = -5.5
SCHR_A = 5.770780            # 4/ln2 (e5m2 has 2 mantissa bits)
SCHR_B = 60.0 + ESHIFT * SCHR_A

C = 256          # input channels
W = 2048         # sequence length
OC = 1024        # conv oc tiles: [conv_out 256 | q 256 | k 256 | v 256]
NH = 8
DKH = 32
QSCALE = float(DKH) ** -0.5
NCT = C // 128   # input-channel tiles (2)
NWT = W // 512   # 512-wide w tiles (4)
NMT = W // 128   # 128-wide w tiles (16)
HB = 36          # head-block stride in vt2 (4-byte aligned)
VW = 384         # vt2 width: 7*HB + 128 window + slack


def build_nc():
    nc = bacc.Bacc("TRN2", target_bir_lowering=False, debug=False)

    x_d = nc.dram_tensor("x", [C, W], F16, kind="ExternalInput")
    wt_d = nc.dram_tensor("wt", [3, C, OC], F16, kind="ExternalInput")     # wt[k,c,oc]
    ball_d = nc.dram_tensor("b_all", [OC], F32, kind="ExternalInput")      # q pre-scaled
    watt_d = nc.dram_tensor("watT", [C, C], F16, kind="ExternalInput")     # w_attn.T
    bat_d = nc.dram_tensor("bat", [C], F32, kind="ExternalInput")
    zz_d = nc.dram_tensor("zz", [4096], U8, kind="ExternalInput")          # zeros
    out_d = nc.dram_tensor("out", [2 * C, W], F32, kind="ExternalOutput")
    # normalized attention staged in [h, w, d] order; the module's faithful
    # (NH,W,dvh)->(256,W) reshape is then a contiguous view of this buffer.
    ahwd_d = nc.dram_tensor("attn_hwd", [NH, W, DKH], F16)

    with tile.TileContext(nc) as tc:
        import contextlib
        with contextlib.ExitStack() as ctx:
            singles = ctx.enter_context(tc.tile_pool(name="singles", bufs=1))
            xp = ctx.enter_context(tc.tile_pool(name="xp", bufs=NCT))
            wtp = ctx.enter_context(tc.tile_pool(name="wtp", bufs=3 * NCT))
            qkp = ctx.enter_context(tc.tile_pool(name="qkp", bufs=4 + NH))
            vtp = ctx.enter_context(tc.tile_pool(name="vtp", bufs=NMT // 2))
            ep = ctx.enter_context(tc.tile_pool(name="ep", bufs=3))
            stage = ctx.enter_context(tc.tile_pool(name="stage", bufs=4))
            norm = ctx.enter_context(tc.tile_pool(name="norm", bufs=10))
            arp = ctx.enter_context(tc.tile_pool(name="arp", bufs=4))

            # ---- constants -------------------------------------------------
            ident16 = singles.tile([128, 128], F16)
            make_identity(nc, ident16[:])
            b_sb = singles.tile([128, 8], F32)  # biases per oc-tile
            nc.gpsimd.dma_start(out=b_sb, in_=ball_d.ap().rearrange("(t p) -> p t", p=128))
            bat_sb = singles.tile([128, 2], F32)
            nc.gpsimd.dma_start(out=bat_sb, in_=bat_d.ap().rearrange("(t p) -> p t", p=128))
            bv_sb = singles.tile([128, C], F32)  # v bias replicated across partitions
            nc.gpsimd.dma_start(
                out=bv_sb, in_=ball_d.ap()[3 * C:4 * C].partition_broadcast(128))
            eshift_sb = singles.tile([128, 1], F32)
            nc.vector.memset(eshift_sb[:], ESHIFT)
            wup = singles.tile([128, 512], F16)  # PE warm-up fodder
            nc.vector.memset(wup[:], 0.0)

            # ---- load x (zero-padded by one column each side) and weights --
            x_sb = []
            for ct in range(NCT):
                t = xp.tile([128, W + 2], F16, tag="x", name=f"x{ct}")
                nc.vector.memset(t[:, 0:1], 0.0)
                nc.vector.memset(t[:, W + 1:W + 2], 0.0)
                nc.gpsimd.dma_start(out=t[:, 1:W + 1], in_=x_d.ap()[ct * 128:(ct + 1) * 128, :])
                x_sb.append(t)
            wt_sb = {}
            wtv_sb = {}
            for kk in range(3):
                for ct in range(NCT):
                    t = wtp.tile([128, OC], F16, tag="wt", name=f"wt{kk}_{ct}")
                    # alternate dispatch queues so the weight loads parallelize
                    # with the x load during the warm-up window
                    eng = nc.sync if (kk + ct) % 2 else nc.gpsimd
                    eng.dma_start(out=t, in_=wt_d.ap()[kk, ct * 128:(ct + 1) * 128, :])
                    wt_sb[kk, ct] = t
                    wtv_sb[kk, ct] = t[:, 3 * C:4 * C]
            watt_sb = []
            for ct in range(NCT):
                t = arp.tile([128, C], F16, tag="watt", name=f"watt{ct}")
                nc.gpsimd.dma_start(out=t, in_=watt_d.ap()[ct * 128:(ct + 1) * 128, :])
                watt_sb.append(t)

            # q8pack[h]: e4m3 [128 qch, 2 planes, W]. Plane n%2 holds head h's
            # 32-channel band for w-chunk n (512 wide); everything else zero.
            # The DoubleRow logits matmul then streams two planes per column
            # slot with exactly one plane non-zero per output column -> real
            # ~1.6x logits speedup, same output layout as fp16.
            q8pack = []
            for h in range(NH):
                t = qkp.tile([128, 2, W], E4, tag="qp", name=f"q8p{h}")
                nc.sync.dma_start(out=t[:].bitcast(U8).rearrange("p a b -> p (a b)"),
                                  in_=zz_d.ap().partition_broadcast(128))
                q8pack.append(t)
            # k8[qt]: e4m3 [128 kch, 2, W], both planes identical copies of k.
            k8 = [qkp.tile([128, 2, W], E4, tag="k8", name=f"k8_{qt}")
                  for qt in range(2)]

            # ---- stage 1: ALL convs (fp16) -------------------------------
            with tc.tile_pool(name="cps", bufs=4, space="PSUM") as cps:
                # PE warm-up burst while the input DMAs run
                wps = cps.tile([128, 512], F32, tag="cps", name="wps")
                for _ in range(20):
                    nc.tensor.matmul(wps[:], wup[:, 0:128], wup[:], start=True, stop=True)

                # vT conv: [w, vch] layout, written as e4m3 (m, m+1) pair
                # tiles; col 32 of each head block = 1.0 (denominator ones).
                vt2 = []
                for mp in range(NMT // 2):
                    vt = vtp.tile([128, 2, VW], E4, tag="vt", name=f"vt{mp}")
                    nc.vector.memset(
                        vt[:, :, 0:NH * HB].rearrange("p a (h e) -> p a h e", e=HB)[:, :, :, 32:36],
                        0.0)
                    nc.vector.memset(
                        vt[:, :, 0:NH * HB].rearrange("p a (h e) -> p a h e", e=HB)[:, :, :, 32:33],
                        1.0)
                    nc.vector.memset(vt[:, :, NH * HB:], 0.0)
                    for s in range(2):
                        m = 2 * mp + s
                        ps = cps.tile([128, C], F32, tag="vps")
                        for ct in range(NCT):
                            for kk in range(3):
                                nc.tensor.matmul(
                                    ps[:],
                                    x_sb[ct][:, m * 128 + kk:m * 128 + kk + 128],
                                    wtv_sb[kk, ct],
                                    start=(ct == 0 and kk == 0),
                                    stop=(ct == NCT - 1 and kk == 2),
                                )
                        nc.vector.tensor_add(
                            vt[:, s, 0:NH * HB].rearrange("p (h e) -> p h e", e=HB)[:, :, 0:32],
                            ps[:].rearrange("p (h d) -> p h d", d=32),
                            bv_sb[:].rearrange("p (h d) -> p h d", d=32),
                        )
                    vt2.append(vt)

                # q/k convs (conv_out is interleaved into stage 2)
                for t in (2, 3, 4, 5):
                    for n in range(NWT):
                        ps = cps.tile([128, 512], F32, tag="cps")
                        for ct in range(NCT):
                            for kk in range(3):
                                nc.tensor.matmul(
                                    ps[:],
                                    wt_sb[kk, ct][:, t * 128:(t + 1) * 128],
                                    x_sb[ct][:, n * 512 + kk:n * 512 + kk + 512],
                                    start=(ct == 0 and kk == 0),
                                    stop=(ct == NCT - 1 and kk == 2),
                                )
                        ns = slice(n * 512, (n + 1) * 512)
                        if t in (2, 3):      # q -> banded fp8 pack, plane n%2
                            for j in range(4):
                                js = slice(32 * j, 32 * j + 32)
                                nc.vector.tensor_scalar_add(
                                    q8pack[4 * (t - 2) + j][js, n % 2, ns],
                                    ps[js, :], b_sb[js, t:t + 1])
                        else:                # k -> both fp8 planes
                            nc.vector.tensor_scalar_add(
                                k8[t - 4][:, :, ns],
                                ps[:].unsqueeze(1).broadcast_to([128, 2, 512]),
                                b_sb[:, t:t + 1])

            # ---- stage 2: attention ---------------------------------------
            with tc.tile_pool(name="lg", bufs=3, space="PSUM") as lg, \
                 tc.tile_pool(name="aps", bufs=2, space="PSUM") as aps:

                ar_sb = [arp.tile([128, W], F16, tag="ar", name=f"ar{i}") for i in range(2)]
                ar = ahwd_d.ap().rearrange("h (g x) d -> (h g) (x d)", g=32)  # [256, 2048]
                pending = []  # normalize work deferred by one half-iteration

                def flush_pending():
                    done_heads = {hh for hh, n, _ in pending if n == 3}
                    for hh, n, a_sb in pending:
                        t4 = lg.tile([128, 4, 34], F16, tag="lg", name=f"t4_{hh}_{n}")
                        for j in range(4):
                            nc.tensor.transpose(
                                t4[:, j, 0:33], a_sb[:, j * 128:(j + 1) * 128],
                                ident16[0:33, 0:33])
                        r4 = norm.tile([128, 4, 1], F16, tag="r", name=f"r{hh}_{n}")
                        with nc.allow_low_precision(reason="softmax recip; 2e-2 tol"):
                            nc.vector.reciprocal(r4[:], t4[:, :, 32:33])
                        z4 = norm.tile([128, 4, 32], F16, tag="z", name=f"z{hh}_{n}")
                        nc.vector.tensor_tensor(
                            out=z4[:], in0=t4[:, :, 0:32],
                            in1=r4[:].broadcast_to([128, 4, 32]),
                            op=mybir.AluOpType.mult)
                        ws = slice(n * 512, (n + 1) * 512)
                        nc.sync.dma_start(
                            out=ahwd_d.ap()[hh, ws, :].rearrange("(j p) d -> p j d", j=4),
                            in_=z4[:])
                    pending.clear()
                    for hh in sorted(done_heads):
                        rr = slice((hh % 4) * 32, (hh % 4) * 32 + 32)
                        nc.sync.dma_start(out=ar_sb[hh // 4][rr, :],
                                          in_=ar[hh * 32:(hh + 1) * 32, :])

                def conv_unit(t, n):
                    # deferred conv unit, hidden under the attention exp
                    # shadow; borrows an lg-pool PSUM buffer briefly
                    ps = lg.tile([128, 512], F32, tag="lg", name=f"co{t}_{n}")
                    for ct in range(NCT):
                        for kk in range(3):
                            nc.tensor.matmul(
                                ps[:],
                                wt_sb[kk, ct][:, t * 128:(t + 1) * 128],
                                x_sb[ct][:, n * 512 + kk:n * 512 + kk + 512],
                                start=(ct == 0 and kk == 0),
                                stop=(ct == NCT - 1 and kk == 2),
                            )
                    ns = slice(n * 512, (n + 1) * 512)
                    co = stage.tile([128, 512], F32, tag="co")
                    nc.scalar.add(co[:], ps[:], b_sb[:, t:t + 1])
                    nc.sync.dma_start(out=out_d.ap()[t * 128:(t + 1) * 128, ns],
                                      in_=co[:])

                for h in range(NH):
                    qt = h // 4
                    for half in range(2):
                        unit = 2 * h + half
                        if unit % 2 == 0:
                            conv_unit((unit // 2) // 4, (unit // 2) % 4)
                        acc = [aps.tile([128, 512], F32, tag="aps",
                                        name=f"acc{h}_{half}_{n2}") for n2 in range(2)]
                        prev = None  # software pipeline: attn trails logits by 1 pair
                        for mp in range(8):
                            e2 = ep.tile([128, 2, 1024], E5, tag="e",
                                         name=f"e{h}_{half}_{mp}")
                            for s in range(2):
                                m = 2 * mp + s
                                ms = slice(m * 128, (m + 1) * 128)
                                lg_t = lg.tile([128, 1024], F32, tag="lg",
                                               name=f"lg{h}_{half}_{m}")
                                for n2 in range(2):
                                    qs = slice(half * 1024 + n2 * 512,
                                               half * 1024 + n2 * 512 + 512)
                                    nc.tensor.matmul(
                                        lg_t[:, n2 * 512:(n2 + 1) * 512],
                                        k8[qt][:, :, ms], q8pack[h][:, :, qs],
                                        start=True, stop=True, perf_mode=DR)
                                # exp split per pair: slot 0 on ScalarE, slot 1
                                # on DVE — the two run in parallel per pair
                                if s == 0:   # exact exp on ScalarE
                                    nc.scalar.activation(
                                        e2[:, s, :], lg_t[:],
                                        mybir.ActivationFunctionType.Exp,
                                        bias=eshift_sb[:])
                                else:        # Schraudolph exp on DVE
                                    nc.vector.tensor_scalar(
                                        out=e2[:, s, :].bitcast(U8), in0=lg_t[:],
                                        scalar1=SCHR_A, scalar2=SCHR_B,
                                        op0=mybir.AluOpType.mult,
                                        op1=mybir.AluOpType.add)
                            if prev is not None:
                                pmp, pe2 = prev
                                for n2 in range(2):
                                    nc.tensor.matmul(
                                        acc[n2][:],
                                        vt2[pmp][:, :, h * HB:h * HB + 128],
                                        pe2[:, :, n2 * 512:(n2 + 1) * 512],
                                        start=(pmp == 0), stop=False,
                                        perf_mode=DR)
                            prev = (mp, e2)
                        pmp, pe2 = prev
                        for n2 in range(2):
                            nc.tensor.matmul(
                                acc[n2][:],
                                vt2[pmp][:, :, h * HB:h * HB + 128],
                                pe2[:, :, n2 * 512:(n2 + 1) * 512],
                                start=False, stop=True,
                                perf_mode=DR)

                        # drain accumulators; defer transposes one half
                        new_pending = []
                        for n2 in range(2):
                            n = 2 * half + n2
                            a_sb = norm.tile([33, 512], F16, tag="asb",
                                             name=f"a{h}_{half}_{n2}")
                            nc.scalar.copy(a_sb[:], acc[n2][0:33, :])
                            new_pending.append((h, n, a_sb))
                        flush_pending()
                        pending = new_pending

                flush_pending()

                # ---- stage 3: 1x1 conv over the (faithful-reshape) view ----
                wk = lg.tile([128, 512], F32, tag="lg", name="warmkeep")
                for _ in range(8):
                    nc.tensor.matmul(wk[:], wup[:, 0:128], wup[:], start=True, stop=True)
                for t2 in range(2):
                    for n in range(NWT):
                        ps = lg.tile([128, 512], F32, tag="lg", name=f"fin{t2}_{n}")
                        for ct in range(NCT):
                            nc.tensor.matmul(
                                ps[:], watt_sb[ct][:, t2 * 128:(t2 + 1) * 128],
                                ar_sb[ct][:, n * 512:(n + 1) * 512],
                                start=(ct == 0), stop=(ct == NCT - 1))
                        fo = stage.tile([128, 512], F32, tag="fo")
                        nc.scalar.add(fo[:], ps[:], bat_sb[:, t2:t2 + 1])
                        nc.sync.dma_start(
                            out=out_d.ap()[C + t2 * 128:C + (t2 + 1) * 128,
                                           n * 512:(n + 1) * 512],
                            in_=fo[:])

    nc.compile()
    return nc


_NC_CACHE = []


def _get_nc():
    if not _NC_CACHE:
        _NC_CACHE.append(build_nc())
    return _NC_CACHE[0]


def _prep_in_maps(x, w_conv, b_conv, w_qkv, b_qkv, w_attn, b_attn):
    x = np.asarray(x, np.float16)
    wq = np.asarray(w_qkv, np.float32).copy()
    wq[0:256] *= QSCALE                                        # fold q scale
    wt = np.ascontiguousarray(
        np.concatenate([np.asarray(w_conv, np.float32), wq], 0)
        .transpose(2, 1, 0).astype(np.float16))                # [3, c, oc]
    bq = np.asarray(b_qkv, np.float32).copy()
    bq[0:256] *= QSCALE
    b_all = np.concatenate([np.asarray(b_conv, np.float32), bq]).copy()
    watt = np.ascontiguousarray(np.asarray(w_attn, np.float32).T.astype(np.float16))
    bat = np.ascontiguousarray(np.asarray(b_attn, np.float32))
    zz = np.zeros(4096, np.uint8)
    return [
        {"x": np.ascontiguousarray(x[b]), "wt": wt, "b_all": b_all,
         "watT": watt, "bat": bat, "zz": zz}
        for b in range(x.shape[0])
    ]


def run(trace=False, **inputs):
    nc = _get_nc()
    in_maps = _prep_in_maps(**inputs)
    res = bass_utils.run_bass_kernel_spmd(
        nc, in_maps, core_ids=list(range(8)), trace=trace,
        **({"trace_cores": [0]} if trace else {}))
    out = np.stack([res.results[i]["out"] for i in range(8)]).astype(np.float32)
    return out, res


def kernel(**inputs) -> np.ndarray:
    out, _ = run(**inputs)
    return out
